# revision 1
# baseline (speedup 1.0000x reference)
"""Multi-head attention (b=2, l=2048, d_model=1024, h=16) on 8 trn2 NeuronCores.

Sharding: tensor-parallel over heads. Each core owns 2 heads: it computes the
QKV projections for its 128 channels (transposed layout), attention for its
heads, and a rank-128 partial of the output projection. The host sums the 8
partials and adds b_o (the tensor-parallel all-reduce, done at gather time).

On-device layout/algorithm per core (all matmuls in float32r, fp32 accumulate):
  warmup:  identity matmul burst to lift the PE HAM clock gate + a dummy exp
           to preload the ACT spline table while input DMAs run.
  phase A: QT/KT/VT [128ch, 4096tok] = W.T @ xT, streamed over 512-token
           chunks; V transposed back to natural [tok, ch] tiles via PE
           transpose, augmented with a ones column (for softmax sums).
  phase B: per (batch, 1024-q-chunk, k-tile): both heads' scoresT[k,q] =
           KT_h'-slice @ QT_h-slice back to back (disjoint PE row groups run
           concurrently); expT = exp(scoresT) on ACT (the phase pacer);
           PV accumulates [V_h | 1].T @ expT into psum [65, 1024] =
           unnormalized attnT plus softmax denominators Z.
  phase C: partial_out[tok, :] = sum_h (attnU_h.T @ Wo_h) * (1/Z_h per token);
           h0/h1 matmuls pair on PE row groups; normalization deferred to
           per-partition scales (ACT) + fused multiply-add (DVE) at PSUM
           evacuation. 1/sqrt(dh) is folded into Wq/bq on the host.
"""
import sys
import types

import numpy as np

D_MODEL = 1024
H = 16
DH = 64
B = 2
L = 2048
BL = B * L            # 4096 tokens
NCORES = 8
NKT = D_MODEL // 128  # 8 feature tiles
TCH = 512             # phase-A token chunk
NCH = BL // TCH       # 8 chunks
QC = 1024             # phase-B q chunk
NQC = L // QC         # 2 per batch
NKB = L // 128        # 16 k-tiles per batch
VSTRIDE = 2 * (DH + 1)  # per-k-tile Vaug columns: [V_h0 | 1 | V_h1 | 1]


def _register_ntff_hook():
    """Install the axon NTFF profiling hook module if the image lacks it.

    Harmless if never used; required for run_bass_kernel_spmd(trace=True)."""
    if "antenv.axon_hooks" in sys.modules:
        return
    try:
        import antenv
        mod = types.ModuleType("antenv.axon_hooks")
        holder = {}
        mod.set_axon_ntff_profile_hook = lambda h: holder.__setitem__("h", h)
        mod.get_axon_ntff_profile_hook = lambda: holder.get("h")
        sys.modules["antenv.axon_hooks"] = mod
        antenv.axon_hooks = mod
        from trn_agent_boot.trn_boot import _ntff_profile_via_ctypes
        mod.set_axon_ntff_profile_hook(
            _ntff_profile_via_ctypes("/opt/axon/libaxon_pjrt.so")
        )
    except Exception:
        pass


_NC_CACHE = {}


def _build():
    if "nc" in _NC_CACHE:
        return _NC_CACHE["nc"]
    import concourse.bacc as bacc
    import concourse.tile as tile
    import concourse.mybir as mybir

    F32 = mybir.dt.float32
    F32R = mybir.dt.float32r
    AF = mybir.ActivationFunctionType
    ALU = mybir.AluOpType

    nc = bacc.Bacc("TRN2", target_bir_lowering=False, debug=False)

    xT_d = nc.dram_tensor("xT", [D_MODEL, BL], F32R, kind="ExternalInput").ap()
    wq_d = nc.dram_tensor("wq", [128, NKT * 128], F32R, kind="ExternalInput").ap()
    wk_d = nc.dram_tensor("wk", [128, NKT * 128], F32R, kind="ExternalInput").ap()
    wv_d = nc.dram_tensor("wv", [128, NKT * 128], F32R, kind="ExternalInput").ap()
    bq_d = nc.dram_tensor("bq", [128, 1], F32, kind="ExternalInput").ap()
    bk_d = nc.dram_tensor("bk", [128, 1], F32, kind="ExternalInput").ap()
    bv_d = nc.dram_tensor("bv", [128, 1], F32, kind="ExternalInput").ap()
    wo_d = nc.dram_tensor("wo", [128, D_MODEL], F32R, kind="ExternalInput").ap()
    id_d = nc.dram_tensor("ident", [128, 128], F32R, kind="ExternalInput").ap()
    out_d = nc.dram_tensor("out", [BL, D_MODEL], F32, kind="ExternalOutput").ap()

    with tile.TileContext(nc) as tc:
        with (
            tc.tile_pool(name="weights", bufs=1) as wpool,
            tc.tile_pool(name="persist", bufs=1) as ppool,
        ):
            id_t = wpool.tile([128, 128], F32R, tag="ident")
            nc.gpsimd.dma_start(id_t[:], id_d)
            wq_t = wpool.tile([128, NKT * 128], F32R, tag="wq")
            wk_t = wpool.tile([128, NKT * 128], F32R, tag="wk")
            wv_t = wpool.tile([128, NKT * 128], F32R, tag="wv")
            bq_t = wpool.tile([128, 1], F32, tag="bq")
            bk_t = wpool.tile([128, 1], F32, tag="bk")
            bv_t = wpool.tile([128, 1], F32, tag="bv")
            wo_t = wpool.tile([128, D_MODEL], F32R, tag="wo")
            for t, d in ((wq_t, wq_d), (wk_t, wk_d), (wv_t, wv_d),
                         (bq_t, bq_d), (bk_t, bk_d), (bv_t, bv_d),
                         (wo_t, wo_d)):
                nc.gpsimd.dma_start(t[:], d)

            QT = ppool.tile([128, BL], F32R, tag="QT")
            KT = ppool.tile([128, BL], F32R, tag="KT")
            VT = ppool.tile([128, BL], F32R, tag="VT")
            Vaug = ppool.tile([128, (BL // 128) * VSTRIDE], F32R, tag="Vaug")
            attnU = [ppool.tile([128, L], F32R, tag=f"attnU{b}",
                                name=f"attnU{b}") for b in range(B)]
            zrow = [[ppool.tile([1, L], F32, tag=f"zrow{h}{b}",
                                name=f"zrow{h}{b}") for b in range(B)]
                    for h in range(2)]
            rz = [[ppool.tile([128, L // 128], F32, tag=f"rz{h}{b}",
                              name=f"rz{h}{b}") for b in range(B)]
                  for h in range(2)]
            scr = ppool.tile([1, 32], F32, tag="scr")

            nc.vector.memset(Vaug[:].bitcast(F32), 1.0)

            # ---- warmup: lift HAM clock gate + preload exp table ----
            with tc.tile_pool(name="psW", bufs=1, space="PSUM") as psW:
                wu = psW.tile([128, 512], F32, tag="wu")
                for i in range(40):
                    nc.tensor.matmul(wu[:, 0:128], id_t[:], id_t[:],
                                     start=(i == 0), stop=(i == 39))
                nc.scalar.activation(scr[:], wu[0:1, 0:32], AF.Exp)

            # ---- phase A: QKV projections (transposed) + V re-transpose ----
            with (
                tc.tile_pool(name="xin", bufs=2) as xpool,
                tc.tile_pool(name="psA", bufs=4, space="PSUM") as psA,
                tc.tile_pool(name="psT", bufs=2, space="PSUM") as psT,
            ):
                for c in range(NCH):
                    sl = slice(c * TCH, (c + 1) * TCH)
                    xt = xpool.tile([128, NKT, TCH], F32R, tag="xchunk")
                    for kt in range(NKT):
                        nc.sync.dma_start(
                            xt[:, kt, :], xT_d[kt * 128:(kt + 1) * 128, sl]
                        )
                    for w_t, b_t, dst in ((wq_t, bq_t, QT), (wk_t, bk_t, KT),
                                          (wv_t, bv_t, VT)):
                        ps = psA.tile([128, TCH], F32, tag="projps")
                        for kt in range(NKT):
                            nc.tensor.matmul(
                                ps[:], w_t[:, kt * 128:(kt + 1) * 128],
                                xt[:, kt, :],
                                start=(kt == 0), stop=(kt == NKT - 1),
                            )
                        nc.vector.tensor_scalar_add(dst[:, sl], ps[:], b_t[:, 0:1])
                    # natural-layout V for the k-tiles this chunk completed
                    for g in range(c * (TCH // 128), (c + 1) * (TCH // 128)):
                        tp = psT.tile([128, 128], F32R, tag="vtrans")
                        nc.tensor.transpose(
                            tp[:], VT[:, g * 128:(g + 1) * 128], id_t[:]
                        )
                        base = g * VSTRIDE
                        nc.vector.tensor_copy(
                            Vaug[:, base:base + DH], tp[:, 0:DH]
                        )
                        nc.vector.tensor_copy(
                            Vaug[:, base + DH + 1:base + 2 * DH + 1],
                            tp[:, DH:2 * DH],
                        )

            # ---- phase B: scoresT -> exp -> PV (ACT paces; PE kept dense) ----
            # One head at a time; scores double-buffered so the PE's runnable
            # window stays deep (enables LDWEIGHTS pull-ahead). A dedicated
            # filler bank takes dep-free full-array matmuls each k-tile so the
            # PE never shows the HAM clock gate an idle window.
            with (
                tc.tile_pool(name="expP", bufs=3) as epool,
                tc.tile_pool(name="att65P", bufs=4) as apool,
                tc.tile_pool(name="oout", bufs=3) as opool,
                tc.tile_pool(name="dram", bufs=1, space="DRAM") as dpool,
                tc.tile_pool(name="psS", bufs=2, space="PSUM") as psS,
                tc.tile_pool(name="psPV", bufs=1, space="PSUM") as psPV,
                tc.tile_pool(name="psO", bufs=1, space="PSUM") as psO,
            ):
                zscr = dpool.tile([2, BL], F32, tag="zscr")

                def emit_filler(pool, tag):
                    """Dep-free full-array matmul: keeps the HAM clock gate
                    open during ACT/evac-paced stretches."""
                    f = pool.tile([128, 512], F32, tag=tag, name="fillt")
                    nc.tensor.matmul(f[:, 0:384], id_t[:], QT[:, 0:384],
                                     start=True, stop=True)

                def emit_c_unit(rc, oc, tail, alt=False):
                    """One output-projection unit: [128 tok, 512] both heads,
                    normalized via deferred per-partition 1/Z scales.

                    During the overlap with attention (tail=False) ACT is busy
                    with exps, so both evacuation ops go to DVE. In the tail,
                    psum tiles alternate into the idle scores/PV slots for
                    pipeline depth, and fillers keep the PE clock gate open."""
                    rsl = slice(rc * 128, (rc + 1) * 128)
                    bi, lrc = rc // (L // 128), rc % (L // 128)
                    lrsl = slice(lrc * 128, (lrc + 1) * 128)
                    osl = slice(oc * 512, (oc + 1) * 512)
                    if alt:
                        ps0 = psS.tile([128, 512], F32, tag="sc", name="ps0a")
                        ps1 = psS.tile([128, 512], F32, tag="sc", name="ps1a")
                    else:
                        ps0 = psO.tile([128, 512], F32, tag="ps0", name="ps0")
                        ps1 = psO.tile([128, 512], F32, tag="ps1", name="ps1")
                    # adjacent pair: row groups 0-63 / 64-127 overlap on PE
                    nc.tensor.matmul(ps0[:], attnU[bi][0:64, lrsl],
                                     wo_t[0:64, osl], start=True, stop=True)
                    nc.tensor.matmul(ps1[:], attnU[bi][64:128, lrsl],
                                     wo_t[64:128, osl], start=True, stop=True)
                    if tail:
                        emit_filler(psPV, "pv")
                    tmp = opool.tile([128, 512], F32, tag="tmp", name="tmp")
                    if tail:
                        nc.scalar.activation(tmp[:], ps0[:], AF.Copy,
                                             scale=rz[0][bi][:, lrc:lrc + 1])
                    else:
                        nc.vector.tensor_scalar_mul(tmp[:], ps0[:],
                                                    rz[0][bi][:, lrc:lrc + 1])
                    ot = opool.tile([128, 512], F32, tag="ot", name="ot")
                    nc.vector.scalar_tensor_tensor(
                        ot[:], ps1[:], rz[1][bi][:, lrc:lrc + 1], tmp[:],
                        op0=ALU.mult, op1=ALU.add,
                    )
                    nc.sync.dma_start(out_d[rsl, osl], ot[:])

                # Output-projection units become PE keep-warm work inside the
                # ACT-paced attention stretches as soon as their inputs exist:
                # batch-0 units during (b1,h0) + (b1,h1,qc0); batch-1's first
                # half during (b1,h1,qc1); only the last 16 run in the tail.
                c_queue = []
                budget = 0.0
                # dense bridge over the phase-A -> B transition: never show
                # the HAM clock gate a low-activity window
                for _ in range(16):
                    emit_filler(psO, "ps0")
                for b in range(B):
                    for h in range(2):
                        hs = slice(h * 64, (h + 1) * 64)
                        for qc in range(NQC):
                            if b == 1 and h == 0 and qc == 0:
                                c_queue += [(rc, oc) for rc in range(16)
                                            for oc in range(2)]
                            if b == 1 and h == 1 and qc == 1:
                                c_queue += [(rc, oc) for rc in range(16, 24)
                                            for oc in range(2)]
                            rate = 1.0
                            q0 = b * L + qc * QC
                            qsl = slice(q0, q0 + QC)
                            pv = psPV.tile([65, QC], F32, tag="pv")
                            for kt in range(NKB):
                                ksl = slice(b * L + kt * 128,
                                            b * L + (kt + 1) * 128)
                                sc = psS.tile([128, QC], F32, tag="sc")
                                for hf in range(QC // 512):
                                    nc.tensor.matmul(
                                        sc[:, hf * 512:(hf + 1) * 512],
                                        KT[hs, ksl],
                                        QT[hs, q0 + hf * 512:q0 + hf * 512 + 512],
                                        start=True, stop=True,
                                    )
                                ex = epool.tile([128, QC], F32R, tag="ex")
                                nc.scalar.activation(ex[:], sc[:], AF.Exp)
                                g = b * NKB + kt
                                vb = g * VSTRIDE + h * (DH + 1)
                                for hf in range(QC // 512):
                                    nc.tensor.matmul(
                                        pv[:, hf * 512:(hf + 1) * 512],
                                        Vaug[:, vb:vb + DH + 1],
                                        ex[:, hf * 512:(hf + 1) * 512],
                                        start=(kt == 0), stop=(kt == NKB - 1),
                                    )
                                if c_queue:
                                    budget += rate
                                    if budget >= 1.0:
                                        budget -= 1.0
                                        emit_c_unit(*c_queue.pop(0), tail=False)
                                    else:
                                        emit_filler(psO, "ps0")
                                else:
                                    emit_filler(psO, "ps0")
                            # bridge the evacuation bubble at the chunk edge
                            for _ in range(8):
                                emit_filler(psO, "ps0")
                            # evacuate: one copy frees the accumulator; the
                            # attnU/Z split happens off the critical path
                            a65 = apool.tile([65, QC], F32R, tag="a65")
                            nc.vector.tensor_copy(a65[:], pv[0:65, :])
                            lqsl = slice(qc * QC, (qc + 1) * QC)
                            nc.vector.tensor_copy(
                                attnU[b][h * 64:(h + 1) * 64, lqsl], a65[0:64, :]
                            )
                            nc.vector.tensor_copy(zrow[h][b][:, lqsl],
                                                  a65[64:65, :])
                            # softmax denominators -> reciprocal columns via
                            # DRAM bounce; per q-chunk on the final stretch so
                            # its output projection can start early
                            zparts = ([lqsl] if (b == 1 and h == 1) or
                                      qc == NQC - 1 else [])
                            if b != 1 or h != 1:
                                zparts = ([slice(0, L)] if qc == NQC - 1 else [])
                            for zsl in zparts:
                                nc.sync.dma_start(
                                    zscr[h:h + 1, b * L + zsl.start:
                                         b * L + zsl.stop],
                                    zrow[h][b][:, zsl])
                                zc = ppool.tile(
                                    [128, (zsl.stop - zsl.start) // 128], F32,
                                    tag=f"zc{h}{b}{qc}", name=f"zc{h}{b}{qc}")
                                nc.sync.dma_start(
                                    zc[:],
                                    zscr[h, b * L + zsl.start:b * L + zsl.stop]
                                    .rearrange("(c p) -> p c", p=128),
                                )
                                nc.vector.reciprocal(
                                    rz[h][b][:, zsl.start // 128:
                                             zsl.stop // 128],
                                    zc[:],
                                )

                # leftover queued units, then the final batch-1 quarter
                c_tail = c_queue + [(rc, oc) for rc in range(24, BL // 128)
                                    for oc in range(2)]
                for i, u in enumerate(c_tail):
                    emit_c_unit(*u, tail=True, alt=(i % 2 == 1))

    nc.compile()
    _NC_CACHE["nc"] = nc
    return nc


def _shard_inputs(x, W_qkv, b_qkv, W_o):
    xT = np.ascontiguousarray(
        x.reshape(BL, D_MODEL).T, dtype=np.float32
    )
    ident = np.eye(128, dtype=np.float32)

    def lhsT_layout(w):
        # [D_MODEL, 128] -> [128, NKT*128] with [p, kt*128+ch] = w[kt*128+p, ch]
        return np.ascontiguousarray(
            w.reshape(NKT, 128, 128).transpose(1, 0, 2).reshape(128, NKT * 128),
            dtype=np.float32,
        )

    in_maps = []
    for c in range(NCORES):
        cs = slice(c * 128, (c + 1) * 128)
        wq = W_qkv[:, cs] * 0.125
        wk = W_qkv[:, D_MODEL:][:, cs]
        wv = W_qkv[:, 2 * D_MODEL:][:, cs]
        in_maps.append({
            "xT": xT,
            "wq": lhsT_layout(wq), "wk": lhsT_layout(wk), "wv": lhsT_layout(wv),
            "bq": np.ascontiguousarray(
                b_qkv[cs] * 0.125, dtype=np.float32).reshape(128, 1),
            "bk": np.ascontiguousarray(
                b_qkv[D_MODEL:][cs], dtype=np.float32).reshape(128, 1),
            "bv": np.ascontiguousarray(
                b_qkv[2 * D_MODEL:][cs], dtype=np.float32).reshape(128, 1),
            "wo": np.ascontiguousarray(W_o[cs, :], dtype=np.float32),
            "ident": ident,
        })
    return in_maps


def _run(inputs, trace=False, tmpdir=None):
    from concourse.bass_utils import run_bass_kernel_spmd

    _register_ntff_hook()
    nc = _build()
    in_maps = _shard_inputs(
        np.asarray(inputs["x"], dtype=np.float32),
        np.asarray(inputs["W_qkv"], dtype=np.float32),
        np.asarray(inputs["b_qkv"], dtype=np.float32),
        np.asarray(inputs["W_o"], dtype=np.float32),
    )
    res = run_bass_kernel_spmd(nc, in_maps, core_ids=list(range(NCORES)),
                               trace=trace, tmpdir=tmpdir)
    partial = np.zeros((BL, D_MODEL), dtype=np.float64)
    for c in range(NCORES):
        partial += res.results[c]["out"].astype(np.float64)
    out = (partial + np.asarray(inputs["b_o"], dtype=np.float64)).astype(np.float32)
    return out.reshape(B, L, D_MODEL), res


def kernel(**inputs) -> np.ndarray:
    out, _ = _run(inputs, trace=False)
    return out



# revision 4
# speedup vs baseline: 1.6940x; 1.6940x over previous
"""Multi-head attention (b=2, l=2048, d_model=1024, h=16) on 8 trn2 NeuronCores.

Sharding: tensor-parallel over heads. Each core owns 2 heads (128 qkv
channels): it computes its QKV projections, attention for its heads, and a
rank-128 partial of the output projection. The host sums the 8 bf16 partials
and adds b_o (the tensor-parallel all-reduce, done at gather time).

v2 design (ACT-paced): all matmul operands bf16 (fp32 psum accumulate).
  phase A: QT/KT [128ch, 4096tok] = W.T @ xT streamed per 512-token chunk;
           V produced directly in natural layout [tok, 130] via xT-stationary
           matmuls against Wv_aug = [V_h0 | 0 | V_h1 | 0] plus a ones-row
           bias matmul that also bakes the softmax ones-columns.
  attention per (b, 512-q-chunk, k-tile): one [128,1024] psum tile holds both
           heads' scoresT (row-group-packed concurrent matmuls); one N=1024
           exp on ACT (the pacer); PV per head accumulates [V_h|1].T @ exp
           into [65, 512] psum over 16 k-tiles (row 64 = softmax denom Z).
  evac per (b, qc): reciprocal(Z) -> gpsimd partition_broadcast -> fused
           multiply: attnU is stored PRE-normalized (bf16), so the output
           projection is a single 128-contraction matmul per [128tok, 512]
           unit, evacuated bf16 and DMA'd out.
  Emission is software-pipelined: phase-A b1 chunks and out-projection units
  are interleaved into the kt-unit stream to fill PE slack under ACT.
"""
import sys
import types

import numpy as np

D_MODEL = 1024
H = 16
DH = 64
B = 2
L = 2048
BL = B * L            # 4096 tokens
NCORES = 8
NKT = D_MODEL // 128  # 8 dmodel tiles
TCH = 512             # phase-A token chunk
NCH = BL // TCH       # 8 chunks
QC = 512              # attention q chunk (per head)
NQC = L // QC         # 4 per batch
NKB = L // 128        # 16 k-tiles per batch
VW = 2 * (DH + 1)     # 130: [V_h0 | 1 | V_h1 | 1]


def _register_ntff_hook():
    """Install the axon NTFF profiling hook module if the image lacks it."""
    if "antenv.axon_hooks" in sys.modules:
        return
    try:
        import antenv
        mod = types.ModuleType("antenv.axon_hooks")
        holder = {}
        mod.set_axon_ntff_profile_hook = lambda h: holder.__setitem__("h", h)
        mod.get_axon_ntff_profile_hook = lambda: holder.get("h")
        sys.modules["antenv.axon_hooks"] = mod
        antenv.axon_hooks = mod
        from trn_agent_boot.trn_boot import _ntff_profile_via_ctypes
        mod.set_axon_ntff_profile_hook(
            _ntff_profile_via_ctypes("/opt/axon/libaxon_pjrt.so")
        )
    except Exception:
        pass


_NC_CACHE = {}


def _build():
    if "nc" in _NC_CACHE:
        return _NC_CACHE["nc"]
    import concourse.bacc as bacc
    import concourse.tile as tile
    import concourse.mybir as mybir

    F32 = mybir.dt.float32
    BF16 = mybir.dt.bfloat16
    AF = mybir.ActivationFunctionType
    ALU = mybir.AluOpType

    nc = bacc.Bacc("TRN2", target_bir_lowering=False, debug=False)

    xT_d = nc.dram_tensor("xT", [D_MODEL, BL], BF16, kind="ExternalInput").ap()
    wq_d = nc.dram_tensor("wq", [128, NKT * 128], BF16, kind="ExternalInput").ap()
    wk_d = nc.dram_tensor("wk", [128, NKT * 128], BF16, kind="ExternalInput").ap()
    wv_d = nc.dram_tensor("wv", [128, NKT, VW], BF16, kind="ExternalInput").ap()
    bq_d = nc.dram_tensor("bq", [128, 1], F32, kind="ExternalInput").ap()
    bk_d = nc.dram_tensor("bk", [128, 1], F32, kind="ExternalInput").ap()
    bv_d = nc.dram_tensor("bv", [1, VW], BF16, kind="ExternalInput").ap()
    wo_d = nc.dram_tensor("wo", [128, D_MODEL], BF16, kind="ExternalInput").ap()
    out_d = nc.dram_tensor("out", [BL, D_MODEL], BF16, kind="ExternalOutput").ap()

    with tile.TileContext(nc) as tc:
        with (
            tc.tile_pool(name="weights", bufs=1) as wpool,
            tc.tile_pool(name="persist", bufs=1) as ppool,
            tc.tile_pool(name="xin", bufs=NCH) as xpool,
            tc.tile_pool(name="expP", bufs=3) as epool,
            tc.tile_pool(name="oout", bufs=3) as opool,
            tc.tile_pool(name="rzP", bufs=2) as rzpool,
            tc.tile_pool(name="zrP", bufs=2) as zrpool,
            tc.tile_pool(name="psS", bufs=2, space="PSUM") as psS,
            tc.tile_pool(name="psPV", bufs=1, space="PSUM") as psPV,
            tc.tile_pool(name="psG", bufs=2, space="PSUM") as psG,
        ):
            # ---- static tiles ----
            wq_t = wpool.tile([128, NKT * 128], BF16, tag="wq")
            wk_t = wpool.tile([128, NKT * 128], BF16, tag="wk")
            wv_t = wpool.tile([128, NKT, VW], BF16, tag="wv")
            bq_t = wpool.tile([128, 1], F32, tag="bq")
            bk_t = wpool.tile([128, 1], F32, tag="bk")
            bv_t = wpool.tile([1, VW], BF16, tag="bv")
            wo_t = wpool.tile([128, D_MODEL], BF16, tag="wo")
            for t, d in ((wq_t, wq_d), (wk_t, wk_d), (wv_t, wv_d),
                         (bq_t, bq_d), (bk_t, bk_d), (bv_t, bv_d),
                         (wo_t, wo_d)):
                nc.gpsimd.dma_start(t[:], d)

            QT = ppool.tile([128, BL], BF16, tag="QT")
            KT = ppool.tile([128, BL], BF16, tag="KT")
            Vaug = ppool.tile([128, B * NKB, VW], BF16, tag="Vaug")
            attnU = [ppool.tile([128, L], BF16, tag=f"attnU{b}",
                                name=f"attnU{b}") for b in range(B)]
            ones_t = ppool.tile([1, 640], BF16, tag="ones")
            scr = ppool.tile([1, 32], F32, tag="scr")
            scrb = ppool.tile([1, 32], BF16, tag="scrb")

            nc.vector.memset(ones_t[:], 1.0)
            nc.vector.memset(scr[:], 0.0)

            # x chunks: all DMAs issued up front (8 bufs, no reuse waits)
            xts = []
            for c in range(NCH):
                xt = xpool.tile([128, NKT, TCH], BF16, tag="xchunk",
                                name=f"x{c}")
                for kt in range(NKT):
                    nc.sync.dma_start(
                        xt[:, kt, :],
                        xT_d[kt * 128:(kt + 1) * 128,
                             c * TCH:(c + 1) * TCH],
                    )
                xts.append(xt)

            # ---- warmup: lift HAM clock gate + preload exp table ----
            wu = psG.tile([128, 512], F32, tag="g", name="warm")
            for i in range(24):
                nc.tensor.matmul(wu[:], ones_t[0:1, 0:128],
                                 ones_t[0:1, 128:640],
                                 start=(i == 0), stop=(i == 23))
            nc.scalar.activation(scrb[:], wu[0:1, 0:32], AF.Exp)

            # ---- emit helpers ----
            def phA_chunk(c):
                """QK projections + natural-layout V for 512 tokens."""
                xt = xts[c]
                for w_t, b_t, dst in ((wq_t, bq_t, QT), (wk_t, bk_t, KT)):
                    ps = psG.tile([128, 512], F32, tag="g", name=f"p{c}")
                    for kt in range(NKT):
                        nc.tensor.matmul(
                            ps[:], w_t[:, kt * 128:(kt + 1) * 128],
                            xt[:, kt, :],
                            start=(kt == 0), stop=(kt == NKT - 1),
                        )
                    nc.vector.tensor_scalar_add(
                        dst[:, c * TCH:(c + 1) * TCH], ps[:], b_t[:, 0:1])
                for tt in range(TCH // 128):
                    g = c * 4 + tt
                    vps = psG.tile([128, 512], F32, tag="g", name=f"v{g}")
                    for kt in range(NKT):
                        nc.tensor.matmul(
                            vps[:, 0:VW],
                            xt[:, kt, tt * 128:(tt + 1) * 128],
                            wv_t[:, kt, :],
                            start=(kt == 0), stop=False,
                        )
                    nc.tensor.matmul(vps[:, 0:VW], ones_t[0:1, 0:128],
                                     bv_t[:], start=False, stop=True)
                    nc.vector.tensor_copy(Vaug[:, g, :], vps[:, 0:VW])

            def att_unit(b, qc, kt, pv0, pv1):
                """Both heads: scoresT -> exp -> PV accumulate, one k-tile."""
                q0 = b * L + qc * QC
                ksl = slice(b * L + kt * 128, b * L + (kt + 1) * 128)
                sc = psS.tile([128, 1024], F32, tag="sc")
                nc.tensor.matmul(sc[:, 0:512], KT[0:64, ksl],
                                 QT[0:64, q0:q0 + QC], start=True, stop=True)
                nc.tensor.matmul(sc[:, 512:1024], KT[64:128, ksl],
                                 QT[64:128, q0:q0 + QC], start=True, stop=True)
                ex = epool.tile([128, 1024], BF16, tag="ex")
                nc.scalar.activation(ex[:], sc[:], AF.Exp)
                g = b * NKB + kt
                nc.tensor.matmul(pv0[:], Vaug[:, g, 0:DH + 1], ex[:, 0:512],
                                 start=(kt == 0), stop=(kt == NKB - 1))
                nc.tensor.matmul(pv1[:], Vaug[:, g, DH + 1:VW],
                                 ex[:, 512:1024],
                                 start=(kt == 0), stop=(kt == NKB - 1))

            def qc_evac(b, qc, pv0, pv1):
                """Z reciprocal -> broadcast -> normalized attnU (bf16)."""
                zr0 = zrpool.tile([1, 512], F32, tag="zr0", name=f"zr0{b}{qc}")
                zr1 = zrpool.tile([1, 512], F32, tag="zr1", name=f"zr1{b}{qc}")
                nc.vector.reciprocal(zr0[:], pv0[64:65, :])
                nc.vector.reciprocal(zr1[:], pv1[64:65, :])
                rzm0 = rzpool.tile([64, 512], F32, tag="rzm0",
                                   name=f"rza{b}{qc}")
                rzm1 = rzpool.tile([64, 512], F32, tag="rzm1",
                                   name=f"rzb{b}{qc}")
                nc.gpsimd.partition_broadcast(rzm0[:], zr0[:])
                nc.gpsimd.partition_broadcast(rzm1[:], zr1[:])
                qsl = slice(qc * QC, (qc + 1) * QC)
                nc.vector.scalar_tensor_tensor(
                    attnU[b][0:64, qsl], pv0[0:64, :], 1.0, rzm0[:],
                    op0=ALU.mult, op1=ALU.mult)
                nc.vector.scalar_tensor_tensor(
                    attnU[b][64:128, qsl], pv1[0:64, :], 1.0, rzm1[:],
                    op0=ALU.mult, op1=ALU.mult)

            def op_unit(b, rc, oc):
                """Output projection for 128 tokens x 512 out-cols."""
                lsl = slice(rc * 128, (rc + 1) * 128)
                rsl = slice(b * L + rc * 128, b * L + (rc + 1) * 128)
                osl = slice(oc * 512, (oc + 1) * 512)
                ps = psG.tile([128, 512], F32, tag="g", name=f"o{b}{rc}{oc}")
                nc.tensor.matmul(ps[:], attnU[b][:, lsl], wo_t[:, osl],
                                 start=True, stop=True)
                ot = opool.tile([128, 512], BF16, tag="ot")
                nc.vector.tensor_copy(ot[:], ps[:])
                nc.sync.dma_start(out_d[rsl, osl], ot[:])

            # ---- software-pipelined emission ----
            # bg queue: thunks interleaved into the kt stream (PE slack work)
            bg = []

            def run_bg(n):
                for _ in range(min(n, len(bg))):
                    bg.pop(0)()

            phA_chunk(0)
            phA_chunk(1)
            for b in range(B):
                for qc in range(NQC):
                    pv0 = psPV.tile([65, 512], F32, tag="pv0",
                                    name=f"pv0_{b}{qc}")
                    pv1 = psPV.tile([65, 512], F32, tag="pv1",
                                    name=f"pv1_{b}{qc}")
                    for kt in range(NKB):
                        if b == 0 and qc == 0 and kt in (4, 8):
                            # chunk kt//4+1 emitted one unit-group ahead
                            phA_chunk(kt // 4 + 1)
                        att_unit(b, qc, kt, pv0, pv1)
                        if kt % 2 == 1:
                            run_bg(1)
                    qc_evac(b, qc, pv0, pv1)
                    # queue this qc's output projection + upcoming phA chunks
                    if b == 0 and qc < 3:
                        bg.append(lambda c=qc + 4: phA_chunk(c))
                    if b == 0 and qc == 3:
                        bg.append(lambda: phA_chunk(7))
                    for rc in range(qc * 4, (qc + 1) * 4):
                        for oc in range(2):
                            bg.append(lambda b=b, rc=rc, oc=oc:
                                      op_unit(b, rc, oc))
            # tail: drain remaining background units
            while bg:
                bg.pop(0)()

    nc.compile()
    _NC_CACHE["nc"] = nc
    return nc


def _shard_inputs(x, W_qkv, b_qkv, W_o):
    import ml_dtypes
    BF = ml_dtypes.bfloat16
    xT = np.ascontiguousarray(
        x.reshape(BL, D_MODEL).T.astype(BF))

    def lhsT_layout(w):
        # [D_MODEL, 128] -> [128, NKT*128] with [p, kt*128+ch] = w[kt*128+p, ch]
        return np.ascontiguousarray(
            w.reshape(NKT, 128, 128).transpose(1, 0, 2)
            .reshape(128, NKT * 128).astype(BF))

    in_maps = []
    for c in range(NCORES):
        cs = slice(c * 128, (c + 1) * 128)
        wq = W_qkv[:, cs] * 0.125
        wk = W_qkv[:, D_MODEL:][:, cs]
        wv = W_qkv[:, 2 * D_MODEL:][:, cs]
        # Wv_aug: [V_h0 | 0 | V_h1 | 0] columns; bias row carries [bv_h0 | 1
        # | bv_h1 | 1] so the ones-row matmul bakes both bias and the softmax
        # ones-columns.
        wv_aug = np.zeros((D_MODEL, VW), dtype=np.float32)
        wv_aug[:, 0:DH] = wv[:, 0:DH]
        wv_aug[:, DH + 1:2 * DH + 1] = wv[:, DH:2 * DH]
        bv = b_qkv[2 * D_MODEL:][cs]
        bv_aug = np.zeros((VW,), dtype=np.float32)
        bv_aug[0:DH] = bv[0:DH]
        bv_aug[DH] = 1.0
        bv_aug[DH + 1:2 * DH + 1] = bv[DH:2 * DH]
        bv_aug[VW - 1] = 1.0
        in_maps.append({
            "xT": xT,
            "wq": lhsT_layout(wq), "wk": lhsT_layout(wk),
            "wv": np.ascontiguousarray(
                wv_aug.reshape(NKT, 128, VW).transpose(1, 0, 2).astype(BF)),
            "bq": np.ascontiguousarray(
                b_qkv[cs] * 0.125, dtype=np.float32).reshape(128, 1),
            "bk": np.ascontiguousarray(
                b_qkv[D_MODEL:][cs], dtype=np.float32).reshape(128, 1),
            "bv": np.ascontiguousarray(bv_aug.astype(BF)).reshape(1, VW),
            "wo": np.ascontiguousarray(W_o[cs, :].astype(BF)),
        })
    return in_maps


def _run(inputs, trace=False, tmpdir=None):
    from concourse.bass_utils import run_bass_kernel_spmd

    _register_ntff_hook()
    nc = _build()
    in_maps = _shard_inputs(
        np.asarray(inputs["x"], dtype=np.float32),
        np.asarray(inputs["W_qkv"], dtype=np.float32),
        np.asarray(inputs["b_qkv"], dtype=np.float32),
        np.asarray(inputs["W_o"], dtype=np.float32),
    )
    res = run_bass_kernel_spmd(nc, in_maps, core_ids=list(range(NCORES)),
                               trace=trace, tmpdir=tmpdir)
    partial = np.zeros((BL, D_MODEL), dtype=np.float32)
    for c in range(NCORES):
        partial += res.results[c]["out"].astype(np.float32)
    out = (partial + np.asarray(inputs["b_o"], dtype=np.float32))
    return out.astype(np.float32).reshape(B, L, D_MODEL), res


def kernel(**inputs) -> np.ndarray:
    out, _ = _run(inputs, trace=False)
    return out


# revision 7
# speedup vs baseline: 1.7657x; 1.0423x over previous
"""Multi-head attention (b=2, l=2048, d_model=1024, h=16) on 8 trn2 NeuronCores.

Sharding: tensor-parallel over heads. Each core owns 2 heads (128 qkv
channels): it computes its QKV projections, attention for its heads, and a
rank-128 partial of the output projection. The host sums the 8 bf16 partials
and adds b_o (the tensor-parallel all-reduce, done at gather time).

v2 design (ACT-paced): all matmul operands bf16 (fp32 psum accumulate).
  phase A: QT/KT [128ch, 4096tok] = W.T @ xT streamed per 512-token chunk;
           V produced directly in natural layout [tok, 130] via xT-stationary
           matmuls against Wv_aug = [V_h0 | 0 | V_h1 | 0] plus a ones-row
           bias matmul that also bakes the softmax ones-columns.
  attention per (b, 512-q-chunk, k-tile): one [128,1024] psum tile holds both
           heads' scoresT (row-group-packed concurrent matmuls); one N=1024
           exp on ACT (the pacer); PV per head accumulates [V_h|1].T @ exp
           into [65, 512] psum over 16 k-tiles (row 64 = softmax denom Z).
  evac per (b, qc): reciprocal(Z) -> gpsimd partition_broadcast -> fused
           multiply: attnU is stored PRE-normalized (bf16), so the output
           projection is a single 128-contraction matmul per [128tok, 512]
           unit, evacuated bf16 and DMA'd out.
  Emission is software-pipelined: phase-A b1 chunks and out-projection units
  are interleaved into the kt-unit stream to fill PE slack under ACT.
"""
import sys
import types

import numpy as np

D_MODEL = 1024
H = 16
DH = 64
B = 2
L = 2048
BL = B * L            # 4096 tokens
NCORES = 8
NKT = D_MODEL // 128  # 8 dmodel tiles
TCH = 512             # phase-A token chunk
NCH = BL // TCH       # 8 chunks
QC = 512              # attention q chunk (per head)
NQC = L // QC         # 4 per batch
NKB = L // 128        # 16 k-tiles per batch
VW = 2 * (DH + 1)     # 130: [V_h0 | 1 | V_h1 | 1]


def _register_ntff_hook():
    """Install the axon NTFF profiling hook module if the image lacks it."""
    if "antenv.axon_hooks" in sys.modules:
        return
    try:
        import antenv
        mod = types.ModuleType("antenv.axon_hooks")
        holder = {}
        mod.set_axon_ntff_profile_hook = lambda h: holder.__setitem__("h", h)
        mod.get_axon_ntff_profile_hook = lambda: holder.get("h")
        sys.modules["antenv.axon_hooks"] = mod
        antenv.axon_hooks = mod
        from trn_agent_boot.trn_boot import _ntff_profile_via_ctypes
        mod.set_axon_ntff_profile_hook(
            _ntff_profile_via_ctypes("/opt/axon/libaxon_pjrt.so")
        )
    except Exception:
        pass


_NC_CACHE = {}


def _build():
    if "nc" in _NC_CACHE:
        return _NC_CACHE["nc"]
    import concourse.bacc as bacc
    import concourse.tile as tile
    import concourse.mybir as mybir

    F32 = mybir.dt.float32
    BF16 = mybir.dt.bfloat16
    AF = mybir.ActivationFunctionType
    ALU = mybir.AluOpType

    nc = bacc.Bacc("TRN2", target_bir_lowering=False, debug=False)

    xT_d = nc.dram_tensor("xT", [D_MODEL, BL], BF16, kind="ExternalInput").ap()
    wq_d = nc.dram_tensor("wq", [128, NKT * 128], BF16, kind="ExternalInput").ap()
    wk_d = nc.dram_tensor("wk", [128, NKT * 128], BF16, kind="ExternalInput").ap()
    wv_d = nc.dram_tensor("wv", [128, NKT, VW], BF16, kind="ExternalInput").ap()
    bq_d = nc.dram_tensor("bq", [128, 1], F32, kind="ExternalInput").ap()
    bk_d = nc.dram_tensor("bk", [128, 1], F32, kind="ExternalInput").ap()
    bv_d = nc.dram_tensor("bv", [1, VW], BF16, kind="ExternalInput").ap()
    wo_d = nc.dram_tensor("wo", [128, D_MODEL], BF16, kind="ExternalInput").ap()
    out_d = nc.dram_tensor("out", [BL, D_MODEL], BF16, kind="ExternalOutput").ap()

    with tile.TileContext(nc) as tc:
        with (
            tc.tile_pool(name="weights", bufs=1) as wpool,
            tc.tile_pool(name="persist", bufs=1) as ppool,
            tc.tile_pool(name="xin", bufs=NCH) as xpool,
            tc.tile_pool(name="expP", bufs=3) as epool,
            tc.tile_pool(name="oout", bufs=3) as opool,
            tc.tile_pool(name="rzP", bufs=2) as rzpool,
            tc.tile_pool(name="zrP", bufs=2) as zrpool,
            tc.tile_pool(name="psS", bufs=2, space="PSUM") as psS,
            tc.tile_pool(name="psPV", bufs=1, space="PSUM") as psPV,
            tc.tile_pool(name="psG", bufs=2, space="PSUM") as psG,
        ):
            # ---- static tiles ----
            wq_t = wpool.tile([128, NKT * 128], BF16, tag="wq")
            wk_t = wpool.tile([128, NKT * 128], BF16, tag="wk")
            wv_t = wpool.tile([128, NKT, VW], BF16, tag="wv")
            bq_t = wpool.tile([128, 1], F32, tag="bq")
            bk_t = wpool.tile([128, 1], F32, tag="bk")
            bv_t = wpool.tile([1, VW], BF16, tag="bv")
            wo_t = wpool.tile([128, D_MODEL], BF16, tag="wo")
            for t, d in ((wq_t, wq_d), (wk_t, wk_d), (wv_t, wv_d),
                         (bq_t, bq_d), (bk_t, bk_d), (bv_t, bv_d),
                         (wo_t, wo_d)):
                nc.gpsimd.dma_start(t[:], d)

            QT = ppool.tile([128, BL], BF16, tag="QT")
            KT = ppool.tile([128, BL], BF16, tag="KT")
            Vaug = ppool.tile([128, B * NKB, VW], BF16, tag="Vaug")
            attnU = [ppool.tile([128, L], BF16, tag=f"attnU{b}",
                                name=f"attnU{b}") for b in range(B)]
            ones_t = ppool.tile([1, 640], BF16, tag="ones")
            scr = ppool.tile([1, 32], F32, tag="scr")
            scrb = ppool.tile([1, 32], BF16, tag="scrb")

            nc.vector.memset(ones_t[:], 1.0)
            nc.vector.memset(scr[:], 0.0)

            # x chunks: one DMA per chunk, all issued up front (8 bufs)
            xts = []
            for c in range(NCH):
                xt = xpool.tile([128, NKT, TCH], BF16, tag="xchunk",
                                name=f"x{c}")
                nc.sync.dma_start(
                    xt[:],
                    xT_d[:, c * TCH:(c + 1) * TCH]
                    .rearrange("(k p) t -> p k t", p=128),
                )
                xts.append(xt)

            # ---- warmup: lift HAM clock gate + preload exp table ----
            wu = psG.tile([128, 512], F32, tag="g", name="warm")
            for i in range(10):
                nc.tensor.matmul(wu[:, 0:128], ones_t[0:1, 0:128],
                                 ones_t[0:1, 128:256],
                                 start=(i == 0), stop=(i == 9))
            nc.scalar.activation(scrb[:], wu[0:1, 0:32], AF.Exp)

            # ---- emit helpers ----
            def phA_qk(c, w_t, b_t, dst, nm):
                """One projection (Q or K) for a 512-token chunk."""
                xt = xts[c]
                ps = psG.tile([128, 512], F32, tag="g", name=f"{nm}{c}")
                for kt in range(NKT):
                    nc.tensor.matmul(
                        ps[:], w_t[:, kt * 128:(kt + 1) * 128],
                        xt[:, kt, :],
                        start=(kt == 0), stop=(kt == NKT - 1),
                    )
                nc.vector.tensor_scalar_add(
                    dst[:, c * TCH:(c + 1) * TCH], ps[:], b_t[:, 0:1])

            def phA_v(c, half):
                """Natural-layout V for 256 tokens (2 token-tiles)."""
                xt = xts[c]
                for tt in (2 * half, 2 * half + 1):
                    g = c * 4 + tt
                    vps = psG.tile([128, 512], F32, tag="g", name=f"v{g}")
                    for kt in range(NKT):
                        nc.tensor.matmul(
                            vps[:, 0:VW],
                            xt[:, kt, tt * 128:(tt + 1) * 128],
                            wv_t[:, kt, :],
                            start=(kt == 0), stop=False,
                        )
                    nc.tensor.matmul(vps[:, 0:VW], ones_t[0:1, 0:128],
                                     bv_t[:], start=False, stop=True)
                    nc.vector.tensor_copy(Vaug[:, g, :], vps[:, 0:VW])

            def sc_exp(b, qc, kt):
                """Both heads' scoresT + exp for one k-tile; returns ex."""
                q0 = b * L + qc * QC
                ksl = slice(b * L + kt * 128, b * L + (kt + 1) * 128)
                sc = psS.tile([128, 1024], F32, tag="sc")
                nc.tensor.matmul(sc[:, 0:512], KT[0:64, ksl],
                                 QT[0:64, q0:q0 + QC], start=True, stop=True)
                nc.tensor.matmul(sc[:, 512:1024], KT[64:128, ksl],
                                 QT[64:128, q0:q0 + QC], start=True, stop=True)
                ex = epool.tile([128, 1024], BF16, tag="ex")
                nc.scalar.activation(ex[:], sc[:], AF.Exp)
                return ex

            def pv_mm(b, kt, ex, pv0, pv1):
                """PV accumulate for one k-tile (lags sc_exp by 2)."""
                g = b * NKB + kt
                nc.tensor.matmul(pv0[:], Vaug[:, g, 0:DH + 1], ex[:, 0:512],
                                 start=(kt == 0), stop=(kt == NKB - 1))
                nc.tensor.matmul(pv1[:], Vaug[:, g, DH + 1:VW],
                                 ex[:, 512:1024],
                                 start=(kt == 0), stop=(kt == NKB - 1))

            def qc_evac(b, qc, pv0, pv1):
                """Z reciprocal -> broadcast -> normalized attnU (bf16)."""
                zr0 = zrpool.tile([1, 512], F32, tag="zr0", name=f"zr0{b}{qc}")
                zr1 = zrpool.tile([1, 512], F32, tag="zr1", name=f"zr1{b}{qc}")
                nc.vector.reciprocal(zr0[:], pv0[64:65, :])
                nc.vector.reciprocal(zr1[:], pv1[64:65, :])
                rzm0 = rzpool.tile([64, 512], F32, tag="rzm0",
                                   name=f"rza{b}{qc}")
                rzm1 = rzpool.tile([64, 512], F32, tag="rzm1",
                                   name=f"rzb{b}{qc}")
                nc.gpsimd.partition_broadcast(rzm0[:], zr0[:])
                nc.gpsimd.partition_broadcast(rzm1[:], zr1[:])
                qsl = slice(qc * QC, (qc + 1) * QC)
                nc.vector.scalar_tensor_tensor(
                    attnU[b][0:64, qsl], pv0[0:64, :], 1.0, rzm0[:],
                    op0=ALU.mult, op1=ALU.mult)
                nc.vector.scalar_tensor_tensor(
                    attnU[b][64:128, qsl], pv1[0:64, :], 1.0, rzm1[:],
                    op0=ALU.mult, op1=ALU.mult)

            def op_unit(b, rc, oc):
                """Output projection for 128 tokens x 512 out-cols."""
                lsl = slice(rc * 128, (rc + 1) * 128)
                rsl = slice(b * L + rc * 128, b * L + (rc + 1) * 128)
                osl = slice(oc * 512, (oc + 1) * 512)
                ps = psG.tile([128, 512], F32, tag="g", name=f"o{b}{rc}{oc}")
                nc.tensor.matmul(ps[:], attnU[b][:, lsl], wo_t[:, osl],
                                 start=True, stop=True)
                ot = opool.tile([128, 512], BF16, tag="ot")
                nc.vector.tensor_copy(ot[:], ps[:])
                nc.sync.dma_start(out_d[rsl, osl], ot[:])

            # ---- software-pipelined emission ----
            # bg FIFO: PE slack work (phase-A quarters, out-proj units)
            # consumed one thunk per kt step. K/V quarters are pushed early
            # (they gate the exp stream); Q quarters deferred (only needed at
            # the matching q-chunk); out-proj units follow their qc_evac.
            bg = []

            def run_bg(n=1):
                for _ in range(min(n, len(bg))):
                    bg.pop(0)()

            def chunk_thunks(c):
                return [
                    lambda: phA_qk(c, wk_t, bk_t, KT, "k"),
                    lambda: phA_v(c, 0),
                    lambda: phA_v(c, 1),
                ]

            # fast start: Q/K of chunk 0 inline; V of chunk 0 + K/V of
            # chunks 1-3 + deferred Qs go through the FIFO
            phA_qk(0, wq_t, bq_t, QT, "q")
            phA_qk(0, wk_t, bk_t, KT, "k")
            bg += [lambda: phA_v(0, 0), lambda: phA_v(0, 1)]
            for c in (1, 2, 3):
                bg += chunk_thunks(c)
            for c in (1, 2, 3):
                bg.append(lambda c=c: phA_qk(c, wq_t, bq_t, QT, "q"))

            for b in range(B):
                for qc in range(NQC):
                    if b == 0 and qc > 0:
                        # one b1 chunk (Q+K+V) per remaining b0 window
                        c = qc + 3
                        bg.insert(0, lambda c=c: phA_qk(c, wq_t, bq_t,
                                                        QT, "q"))
                        bg[1:1] = chunk_thunks(c)
                    if b == 1 and qc == 0:
                        # chunk 7 K/V lead the b1-qc0 window (needed by kt12)
                        bg[0:0] = chunk_thunks(7)
                    if b == 1 and qc == 1:
                        bg.insert(0, lambda: phA_qk(7, wq_t, bq_t, QT, "q"))
                    pv0 = psPV.tile([65, 512], F32, tag="pv0",
                                    name=f"pv0_{b}{qc}")
                    pv1 = psPV.tile([65, 512], F32, tag="pv1",
                                    name=f"pv1_{b}{qc}")
                    exq = {}
                    for kt in range(NKB):
                        exq[kt] = sc_exp(b, qc, kt)
                        if kt >= 2:
                            pv_mm(b, kt - 2, exq.pop(kt - 2), pv0, pv1)
                        run_bg(1)
                    pv_mm(b, NKB - 2, exq.pop(NKB - 2), pv0, pv1)
                    pv_mm(b, NKB - 1, exq.pop(NKB - 1), pv0, pv1)
                    qc_evac(b, qc, pv0, pv1)
                    for rc in range(qc * 4, (qc + 1) * 4):
                        for oc in range(2):
                            bg.append(lambda b=b, rc=rc, oc=oc:
                                      op_unit(b, rc, oc))
            # tail: drain remaining background units
            while bg:
                bg.pop(0)()

    nc.compile()
    _NC_CACHE["nc"] = nc
    return nc


def _shard_inputs(x, W_qkv, b_qkv, W_o):
    import ml_dtypes
    BF = ml_dtypes.bfloat16
    xT = np.ascontiguousarray(
        x.reshape(BL, D_MODEL).T.astype(BF))

    def lhsT_layout(w):
        # [D_MODEL, 128] -> [128, NKT*128] with [p, kt*128+ch] = w[kt*128+p, ch]
        return np.ascontiguousarray(
            w.reshape(NKT, 128, 128).transpose(1, 0, 2)
            .reshape(128, NKT * 128).astype(BF))

    in_maps = []
    for c in range(NCORES):
        cs = slice(c * 128, (c + 1) * 128)
        wq = W_qkv[:, cs] * 0.125
        wk = W_qkv[:, D_MODEL:][:, cs]
        wv = W_qkv[:, 2 * D_MODEL:][:, cs]
        # Wv_aug: [V_h0 | 0 | V_h1 | 0] columns; bias row carries [bv_h0 | 1
        # | bv_h1 | 1] so the ones-row matmul bakes both bias and the softmax
        # ones-columns.
        wv_aug = np.zeros((D_MODEL, VW), dtype=np.float32)
        wv_aug[:, 0:DH] = wv[:, 0:DH]
        wv_aug[:, DH + 1:2 * DH + 1] = wv[:, DH:2 * DH]
        bv = b_qkv[2 * D_MODEL:][cs]
        bv_aug = np.zeros((VW,), dtype=np.float32)
        bv_aug[0:DH] = bv[0:DH]
        bv_aug[DH] = 1.0
        bv_aug[DH + 1:2 * DH + 1] = bv[DH:2 * DH]
        bv_aug[VW - 1] = 1.0
        in_maps.append({
            "xT": xT,
            "wq": lhsT_layout(wq), "wk": lhsT_layout(wk),
            "wv": np.ascontiguousarray(
                wv_aug.reshape(NKT, 128, VW).transpose(1, 0, 2).astype(BF)),
            "bq": np.ascontiguousarray(
                b_qkv[cs] * 0.125, dtype=np.float32).reshape(128, 1),
            "bk": np.ascontiguousarray(
                b_qkv[D_MODEL:][cs], dtype=np.float32).reshape(128, 1),
            "bv": np.ascontiguousarray(bv_aug.astype(BF)).reshape(1, VW),
            "wo": np.ascontiguousarray(W_o[cs, :].astype(BF)),
        })
    return in_maps


def _run(inputs, trace=False, tmpdir=None):
    from concourse.bass_utils import run_bass_kernel_spmd

    _register_ntff_hook()
    nc = _build()
    in_maps = _shard_inputs(
        np.asarray(inputs["x"], dtype=np.float32),
        np.asarray(inputs["W_qkv"], dtype=np.float32),
        np.asarray(inputs["b_qkv"], dtype=np.float32),
        np.asarray(inputs["W_o"], dtype=np.float32),
    )
    res = run_bass_kernel_spmd(nc, in_maps, core_ids=list(range(NCORES)),
                               trace=trace, tmpdir=tmpdir)
    partial = np.zeros((BL, D_MODEL), dtype=np.float32)
    for c in range(NCORES):
        partial += res.results[c]["out"].astype(np.float32)
    out = (partial + np.asarray(inputs["b_o"], dtype=np.float32))
    return out.astype(np.float32).reshape(B, L, D_MODEL), res


def kernel(**inputs) -> np.ndarray:
    out, _ = _run(inputs, trace=False)
    return out


# revision 11
# speedup vs baseline: 1.7740x; 1.0047x over previous
"""Multi-head attention (b=2, l=2048, d_model=1024, h=16) on 8 trn2 NeuronCores.

Sharding: tensor-parallel over heads. Each core owns 2 heads (128 qkv
channels): it computes its QKV projections, attention for its heads, and a
rank-128 partial of the output projection. The host sums the 8 bf16 partials
and adds b_o (the tensor-parallel all-reduce, done at gather time).

v2 design (ACT-paced): all matmul operands bf16 (fp32 psum accumulate).
  phase A: QT/KT [128ch, 4096tok] = W.T @ xT streamed per 512-token chunk;
           V produced directly in natural layout [tok, 130] via xT-stationary
           matmuls against Wv_aug = [V_h0 | 0 | V_h1 | 0] plus a ones-row
           bias matmul that also bakes the softmax ones-columns.
  attention per (b, 512-q-chunk, k-tile): one [128,1024] psum tile holds both
           heads' scoresT (row-group-packed concurrent matmuls); one N=1024
           exp on ACT (the pacer); PV per head accumulates [V_h|1].T @ exp
           into [65, 512] psum over 16 k-tiles (row 64 = softmax denom Z).
  evac per (b, qc): reciprocal(Z) -> gpsimd partition_broadcast -> fused
           multiply: attnU is stored PRE-normalized (bf16), so the output
           projection is a single 128-contraction matmul per [128tok, 512]
           unit, evacuated bf16 and DMA'd out.
  Emission is software-pipelined: phase-A b1 chunks and out-projection units
  are interleaved into the kt-unit stream to fill PE slack under ACT.
"""
import sys
import types

import numpy as np

D_MODEL = 1024
H = 16
DH = 64
B = 2
L = 2048
BL = B * L            # 4096 tokens
NCORES = 8
NKT = D_MODEL // 128  # 8 dmodel tiles
TCH = 512             # phase-A token chunk
NCH = BL // TCH       # 8 chunks
QC = 512              # attention q chunk (per head)
NQC = L // QC         # 4 per batch
NKB = L // 128        # 16 k-tiles per batch
VW = 2 * (DH + 1)     # 130: [V_h0 | 1 | V_h1 | 1]


def _register_ntff_hook():
    """Install the axon NTFF profiling hook module if the image lacks it."""
    if "antenv.axon_hooks" in sys.modules:
        return
    try:
        import antenv
        mod = types.ModuleType("antenv.axon_hooks")
        holder = {}
        mod.set_axon_ntff_profile_hook = lambda h: holder.__setitem__("h", h)
        mod.get_axon_ntff_profile_hook = lambda: holder.get("h")
        sys.modules["antenv.axon_hooks"] = mod
        antenv.axon_hooks = mod
        from trn_agent_boot.trn_boot import _ntff_profile_via_ctypes
        mod.set_axon_ntff_profile_hook(
            _ntff_profile_via_ctypes("/opt/axon/libaxon_pjrt.so")
        )
    except Exception:
        pass


_NC_CACHE = {}


def _build():
    if "nc" in _NC_CACHE:
        return _NC_CACHE["nc"]
    import concourse.bacc as bacc
    import concourse.tile as tile
    import concourse.mybir as mybir

    F32 = mybir.dt.float32
    BF16 = mybir.dt.bfloat16
    AF = mybir.ActivationFunctionType
    ALU = mybir.AluOpType

    nc = bacc.Bacc("TRN2", target_bir_lowering=False, debug=False)

    xT_d = nc.dram_tensor("xT", [D_MODEL, BL], BF16, kind="ExternalInput").ap()
    wq_d = nc.dram_tensor("wq", [128, NKT * 128], BF16, kind="ExternalInput").ap()
    wk_d = nc.dram_tensor("wk", [128, NKT * 128], BF16, kind="ExternalInput").ap()
    wv_d = nc.dram_tensor("wv", [128, NKT, VW], BF16, kind="ExternalInput").ap()
    bq_d = nc.dram_tensor("bq", [128, 1], F32, kind="ExternalInput").ap()
    bk_d = nc.dram_tensor("bk", [128, 1], F32, kind="ExternalInput").ap()
    bv_d = nc.dram_tensor("bv", [1, VW], BF16, kind="ExternalInput").ap()
    wo_d = nc.dram_tensor("wo", [128, D_MODEL], BF16, kind="ExternalInput").ap()
    out_d = nc.dram_tensor("out", [BL, D_MODEL], BF16, kind="ExternalOutput").ap()

    with tile.TileContext(nc) as tc:
        with (
            tc.tile_pool(name="weights", bufs=1) as wpool,
            tc.tile_pool(name="persist", bufs=1) as ppool,
            tc.tile_pool(name="xin", bufs=NCH) as xpool,
            tc.tile_pool(name="expP", bufs=3) as epool,
            tc.tile_pool(name="oout", bufs=3) as opool,
            tc.tile_pool(name="rzP", bufs=2) as rzpool,
            tc.tile_pool(name="zrP", bufs=2) as zrpool,
            tc.tile_pool(name="psS", bufs=2, space="PSUM") as psS,
            tc.tile_pool(name="psPV", bufs=1, space="PSUM") as psPV,
            tc.tile_pool(name="psG", bufs=2, space="PSUM") as psG,
        ):
            # ---- static tiles ----
            wq_t = wpool.tile([128, NKT * 128], BF16, tag="wq")
            wk_t = wpool.tile([128, NKT * 128], BF16, tag="wk")
            wv_t = wpool.tile([128, NKT, VW], BF16, tag="wv")
            bq_t = wpool.tile([128, 1], F32, tag="bq")
            bk_t = wpool.tile([128, 1], F32, tag="bk")
            bv_t = wpool.tile([1, VW], BF16, tag="bv")
            wo_t = wpool.tile([128, D_MODEL], BF16, tag="wo")
            for t, d in ((wq_t, wq_d), (wk_t, wk_d), (wv_t, wv_d),
                         (bq_t, bq_d), (bk_t, bk_d), (bv_t, bv_d),
                         (wo_t, wo_d)):
                nc.gpsimd.dma_start(t[:], d)

            QT = ppool.tile([128, BL], BF16, tag="QT")
            KT = ppool.tile([128, BL], BF16, tag="KT")
            Vaug = ppool.tile([128, B * NKB, VW], BF16, tag="Vaug")
            attnU = [ppool.tile([128, L], BF16, tag=f"attnU{b}",
                                name=f"attnU{b}") for b in range(B)]
            ones_t = ppool.tile([1, 640], BF16, tag="ones")
            scr = ppool.tile([1, 32], F32, tag="scr")
            scrb = ppool.tile([1, 32], BF16, tag="scrb")

            nc.vector.memset(ones_t[:], 1.0)
            nc.vector.memset(scr[:], 0.0)

            # x chunks: one DMA per chunk, all issued up front (8 bufs)
            xts = []
            for c in range(NCH):
                xt = xpool.tile([128, NKT, TCH], BF16, tag="xchunk",
                                name=f"x{c}")
                nc.sync.dma_start(
                    xt[:],
                    xT_d[:, c * TCH:(c + 1) * TCH]
                    .rearrange("(k p) t -> p k t", p=128),
                )
                xts.append(xt)

            # ---- warmup: lift HAM clock gate + preload exp table ----
            # ~3.5us of continuous matmuls so the PE is at K=8/8 before
            # phase A starts; the exp preloads the ACT spline table.
            wu = psG.tile([128, 512], F32, tag="g", name="warm")
            for i in range(36):
                nc.tensor.matmul(wu[:, 0:128], ones_t[0:1, 0:128],
                                 ones_t[0:1, 128:256],
                                 start=(i == 0), stop=(i == 35))
            nc.scalar.activation(scrb[:], wu[0:1, 0:32], AF.Exp)

            # ---- emit helpers ----
            psq = {}

            def phA_qk(c, w_t, b_t, dst, nm, half):
                """Half a projection (4 k-tiles) for a 512-token chunk.

                Split so each bg thunk stays under ~1us of PE time; the two
                halves share one psum accumulator stashed in psq.
                """
                xt = xts[c]
                if half == 0:
                    ps = psq[(nm, c)] = psG.tile([128, 512], F32, tag="g",
                                                 name=f"{nm}{c}")
                else:
                    ps = psq.pop((nm, c))
                for kt in range(4 * half, 4 * half + 4):
                    nc.tensor.matmul(
                        ps[:], w_t[:, kt * 128:(kt + 1) * 128],
                        xt[:, kt, :],
                        start=(kt == 0), stop=(kt == NKT - 1),
                    )
                if half == 1:
                    nc.vector.tensor_scalar_add(
                        dst[:, c * TCH:(c + 1) * TCH], ps[:], b_t[:, 0:1])

            def phA_v(c, half):
                """Natural-layout V for 256 tokens (2 token-tiles)."""
                xt = xts[c]
                for tt in (2 * half, 2 * half + 1):
                    g = c * 4 + tt
                    vps = psG.tile([128, 512], F32, tag="g", name=f"v{g}")
                    for kt in range(NKT):
                        nc.tensor.matmul(
                            vps[:, 0:VW],
                            xt[:, kt, tt * 128:(tt + 1) * 128],
                            wv_t[:, kt, :],
                            start=(kt == 0), stop=False,
                        )
                    nc.tensor.matmul(vps[:, 0:VW], ones_t[0:1, 0:128],
                                     bv_t[:], start=False, stop=True)
                    nc.vector.tensor_copy(Vaug[:, g, :], vps[:, 0:VW])

            def sc_exp(b, qc, kt):
                """Both heads' scoresT + exp for one k-tile; returns ex."""
                q0 = b * L + qc * QC
                ksl = slice(b * L + kt * 128, b * L + (kt + 1) * 128)
                sc = psS.tile([128, 1024], F32, tag="sc")
                nc.tensor.matmul(sc[:, 0:512], KT[0:64, ksl],
                                 QT[0:64, q0:q0 + QC], start=True, stop=True)
                nc.tensor.matmul(sc[:, 512:1024], KT[64:128, ksl],
                                 QT[64:128, q0:q0 + QC], start=True, stop=True)
                ex = epool.tile([128, 1024], BF16, tag="ex")
                nc.scalar.activation(ex[:], sc[:], AF.Exp)
                return ex

            def pv_mm(b, kt, ex, pv0, pv1):
                """PV accumulate for one k-tile (lags sc_exp by 2)."""
                g = b * NKB + kt
                nc.tensor.matmul(pv0[:], Vaug[:, g, 0:DH + 1], ex[:, 0:512],
                                 start=(kt == 0), stop=(kt == NKB - 1))
                nc.tensor.matmul(pv1[:], Vaug[:, g, DH + 1:VW],
                                 ex[:, 512:1024],
                                 start=(kt == 0), stop=(kt == NKB - 1))

            def qc_evac(b, qc, pv0, pv1):
                """Stage pv out of psum fast, then Z reciprocal ->
                broadcast -> normalized attnU (bf16) off the staging copy.

                The two stage copies free the psum accumulators in ~1.5us so
                the next q-chunk's PV can start; the slower recip/broadcast
                chain runs concurrently with the resumed pipeline."""
                st0 = rzpool.tile([65, 512], F32, tag="st0",
                                  name=f"st0{b}{qc}")
                st1 = rzpool.tile([65, 512], F32, tag="st1",
                                  name=f"st1{b}{qc}")
                nc.vector.tensor_copy(st0[:], pv0[:])
                nc.vector.tensor_copy(st1[:], pv1[:])
                zr0 = zrpool.tile([1, 512], F32, tag="zr0", name=f"zr0{b}{qc}")
                zr1 = zrpool.tile([1, 512], F32, tag="zr1", name=f"zr1{b}{qc}")
                nc.vector.reciprocal(zr0[:], st0[64:65, :])
                nc.vector.reciprocal(zr1[:], st1[64:65, :])
                rzm0 = rzpool.tile([64, 512], F32, tag="rzm0",
                                   name=f"rza{b}{qc}")
                rzm1 = rzpool.tile([64, 512], F32, tag="rzm1",
                                   name=f"rzb{b}{qc}")
                nc.gpsimd.partition_broadcast(rzm0[:], zr0[:])
                nc.gpsimd.partition_broadcast(rzm1[:], zr1[:])
                qsl = slice(qc * QC, (qc + 1) * QC)
                nc.vector.scalar_tensor_tensor(
                    attnU[b][0:64, qsl], st0[0:64, :], 1.0, rzm0[:],
                    op0=ALU.mult, op1=ALU.mult)
                nc.vector.scalar_tensor_tensor(
                    attnU[b][64:128, qsl], st1[0:64, :], 1.0, rzm1[:],
                    op0=ALU.mult, op1=ALU.mult)

            def op_unit(b, rc, oc):
                """Output projection for 128 tokens x 512 out-cols."""
                lsl = slice(rc * 128, (rc + 1) * 128)
                rsl = slice(b * L + rc * 128, b * L + (rc + 1) * 128)
                osl = slice(oc * 512, (oc + 1) * 512)
                ps = psG.tile([128, 512], F32, tag="g", name=f"o{b}{rc}{oc}")
                nc.tensor.matmul(ps[:], attnU[b][:, lsl], wo_t[:, osl],
                                 start=True, stop=True)
                ot = opool.tile([128, 512], BF16, tag="ot")
                nc.vector.tensor_copy(ot[:], ps[:])
                nc.sync.dma_start(out_d[rsl, osl], ot[:])

            # ---- software-pipelined emission ----
            # bg FIFO: PE slack work (phase-A halves, out-proj units)
            # consumed one thunk per kt step, placed so each chunk's K/V
            # lands before the kt that needs it and each window stays under
            # the ACT budget. QK halves must occupy ADJACENT slots (they
            # share a rotating psum accumulator).
            bg = []

            def run_bg(n=1):
                for _ in range(min(n, len(bg))):
                    bg.pop(0)()

            def qh(c, h, w_t=None, b_t=None, dst=None, nm=None):
                if w_t is None:
                    w_t, b_t, dst, nm = wq_t, bq_t, QT, "q"
                return lambda: phA_qk(c, w_t, b_t, dst, nm, h)

            def kh(c, h):
                return qh(c, h, wk_t, bk_t, KT, "k")

            def vh(c, h):
                return lambda: phA_v(c, h)

            # fast start: Q/K of chunk 0 inline (unsplit emission order)
            phA_qk(0, wq_t, bq_t, QT, "q", 0)
            phA_qk(0, wq_t, bq_t, QT, "q", 1)
            phA_qk(0, wk_t, bk_t, KT, "k", 0)
            phA_qk(0, wk_t, bk_t, KT, "k", 1)
            # per-window background schedules (deadline-ordered)
            sched = {
                (0, 0): [vh(0, 0), vh(0, 1), kh(1, 0), kh(1, 1),
                         vh(1, 0), vh(1, 1), kh(2, 0), kh(2, 1),
                         vh(2, 0), vh(2, 1), kh(3, 0), kh(3, 1),
                         vh(3, 0), vh(3, 1), qh(1, 0), qh(1, 1)],
                (0, 1): [qh(2, 0), qh(2, 1), qh(4, 0), qh(4, 1),
                         kh(4, 0), kh(4, 1), vh(4, 0), vh(4, 1)],
                (0, 2): [qh(3, 0), qh(3, 1), qh(5, 0), qh(5, 1),
                         kh(5, 0), kh(5, 1), vh(5, 0), vh(5, 1)],
                (0, 3): [qh(6, 0), qh(6, 1), kh(6, 0), kh(6, 1),
                         vh(6, 0), vh(6, 1)],
                (1, 0): [kh(7, 0), kh(7, 1), vh(7, 0), vh(7, 1)],
                (1, 1): [qh(7, 0), qh(7, 1)],
            }

            for b in range(B):
                for qc in range(NQC):
                    bg = sched.get((b, qc), []) + bg
                    pv0 = psPV.tile([65, 512], F32, tag="pv0",
                                    name=f"pv0_{b}{qc}")
                    pv1 = psPV.tile([65, 512], F32, tag="pv1",
                                    name=f"pv1_{b}{qc}")
                    exq = {}
                    for kt in range(NKB):
                        exq[kt] = sc_exp(b, qc, kt)
                        if kt >= 2:
                            pv_mm(b, kt - 2, exq.pop(kt - 2), pv0, pv1)
                        run_bg(1)
                    pv_mm(b, NKB - 2, exq.pop(NKB - 2), pv0, pv1)
                    pv_mm(b, NKB - 1, exq.pop(NKB - 1), pv0, pv1)
                    qc_evac(b, qc, pv0, pv1)
                    for rc in range(qc * 4, (qc + 1) * 4):
                        for oc in range(2):
                            bg.append(lambda b=b, rc=rc, oc=oc:
                                      op_unit(b, rc, oc))
            # tail: drain remaining background units
            while bg:
                bg.pop(0)()

    nc.compile()
    _NC_CACHE["nc"] = nc
    return nc


def _shard_inputs(x, W_qkv, b_qkv, W_o):
    import ml_dtypes
    BF = ml_dtypes.bfloat16
    xT = np.ascontiguousarray(
        x.reshape(BL, D_MODEL).T.astype(BF))

    def lhsT_layout(w):
        # [D_MODEL, 128] -> [128, NKT*128] with [p, kt*128+ch] = w[kt*128+p, ch]
        return np.ascontiguousarray(
            w.reshape(NKT, 128, 128).transpose(1, 0, 2)
            .reshape(128, NKT * 128).astype(BF))

    in_maps = []
    for c in range(NCORES):
        cs = slice(c * 128, (c + 1) * 128)
        wq = W_qkv[:, cs] * 0.125
        wk = W_qkv[:, D_MODEL:][:, cs]
        wv = W_qkv[:, 2 * D_MODEL:][:, cs]
        # Wv_aug: [V_h0 | 0 | V_h1 | 0] columns; bias row carries [bv_h0 | 1
        # | bv_h1 | 1] so the ones-row matmul bakes both bias and the softmax
        # ones-columns.
        wv_aug = np.zeros((D_MODEL, VW), dtype=np.float32)
        wv_aug[:, 0:DH] = wv[:, 0:DH]
        wv_aug[:, DH + 1:2 * DH + 1] = wv[:, DH:2 * DH]
        bv = b_qkv[2 * D_MODEL:][cs]
        bv_aug = np.zeros((VW,), dtype=np.float32)
        bv_aug[0:DH] = bv[0:DH]
        bv_aug[DH] = 1.0
        bv_aug[DH + 1:2 * DH + 1] = bv[DH:2 * DH]
        bv_aug[VW - 1] = 1.0
        in_maps.append({
            "xT": xT,
            "wq": lhsT_layout(wq), "wk": lhsT_layout(wk),
            "wv": np.ascontiguousarray(
                wv_aug.reshape(NKT, 128, VW).transpose(1, 0, 2).astype(BF)),
            "bq": np.ascontiguousarray(
                b_qkv[cs] * 0.125, dtype=np.float32).reshape(128, 1),
            "bk": np.ascontiguousarray(
                b_qkv[D_MODEL:][cs], dtype=np.float32).reshape(128, 1),
            "bv": np.ascontiguousarray(bv_aug.astype(BF)).reshape(1, VW),
            "wo": np.ascontiguousarray(W_o[cs, :].astype(BF)),
        })
    return in_maps


def _run(inputs, trace=False, tmpdir=None):
    from concourse.bass_utils import run_bass_kernel_spmd

    _register_ntff_hook()
    nc = _build()
    in_maps = _shard_inputs(
        np.asarray(inputs["x"], dtype=np.float32),
        np.asarray(inputs["W_qkv"], dtype=np.float32),
        np.asarray(inputs["b_qkv"], dtype=np.float32),
        np.asarray(inputs["W_o"], dtype=np.float32),
    )
    res = run_bass_kernel_spmd(nc, in_maps, core_ids=list(range(NCORES)),
                               trace=trace, tmpdir=tmpdir)
    partial = np.zeros((BL, D_MODEL), dtype=np.float32)
    for c in range(NCORES):
        partial += res.results[c]["out"].astype(np.float32)
    out = (partial + np.asarray(inputs["b_o"], dtype=np.float32))
    return out.astype(np.float32).reshape(B, L, D_MODEL), res


def kernel(**inputs) -> np.ndarray:
    out, _ = _run(inputs, trace=False)
    return out
